# revision 1
# baseline (speedup 1.0000x reference)
"""DualAttentionAutoEncoder (DA-RNN) Trainium2 kernel.

Pure data parallel over 8 NeuronCores: batch 8192 -> 1024 rows/core; the
full (unsharded) inputs are sharded on host, one identical NEFF runs SPMD
on cores 0-7, outputs are concatenated.

Algebraic structure exploited:
  * The encoder input attention is softmax-shift-invariant: the
    (h@wh + c@wc) term is constant across the softmax axis, so the
    attention weights are independent of the recurrence and constant in
    time: at = softmax_d(score_x). All wi_t = at*x_t and their Wih
    projections are precomputed outside the recurrence; the encoder loop
    is a plain LSTM.
  * sigmoid(x) = (tanh(x/2)+1)/2, and the g-gate weights are pre-scaled
    by 2 on host, so one tanh(0.5*x) ACT op covers all 4 gates (single
    exp+tanh activation-table set; no table switches).
  * LSTM gate biases are folded into the gate-tanh Activation's
    per-partition bias operand (gates live on partitions), removing the
    bias rows from the matmuls and the per-step y_t staging DMAs.
  * Decoder y_tilde fc-layer folds into the LSTM gate matmul:
    Wih@(fc_w@[ctx;y]+fc_b)+b = W2c@ctx + W2y@y + b' (host-prepped).
  * Decoder temporal-softmax bias b2 is shift-invariant -> dropped.
  * score_x is computed as PE-accumulated scaled-identity matmuls.

Implementation notes:
  * Everything on the recurrent path is bf16 (h, c, x_encT, weights):
    DVE gets 2x elementwise throughput on packed bf16, PE runs 1
    cycle/row, and the 2e-2 rel-err budget has >10x headroom.
  * Per-step work is split into 2 batch-halves emitted stage-major so
    the in-order engine queues pipeline the halves; elementwise ops are
    emitted at the widest width the dependency structure allows (per-op
    fixed overheads are ~60ns DVE / ~185ns ACT).
  * PSUM->SBUF copies run on the otherwise-idle Pool (gpsimd) engine.
  * The decoder attention ctx = sum_l at_l * x_enc_l is a bf16
    multiply + fold tree (10->5->2->1 lags) in batch-major layout,
    then a PE transpose back to feature-major.
  * y_hist is staged as yT5 [5, 10*B] (lag-major columns) so the
    decoder gate matmul reads y_t at partition base 0 every step.
"""

import os
import sys

for _p in ("/opt/trn_rl_repo", "/root/.axon_site/_ro/trn_rl_repo"):
    if os.path.isdir(_p) and _p not in sys.path:
        sys.path.insert(0, _p)

import numpy as np
import ml_dtypes

import concourse.bass as bass
import concourse.bacc as bacc
import concourse.mybir as mybir
import concourse.tile as tile

F32 = mybir.dt.float32
F32R = mybir.dt.float32r
BF16 = mybir.dt.bfloat16
FP32 = np.float32
AF = mybir.ActivationFunctionType
ALU = mybir.AluOpType
AX = mybir.AxisListType

H, L, D, OUT = 64, 10, 128, 5
B_FULL = 8192
NCORES = 8


def _prep_weights(inp):
    """Host-side (numpy) preparation of the tiny weight tensors."""
    f = lambda a: np.ascontiguousarray(a, dtype=FP32)
    bf = lambda a: np.ascontiguousarray(np.asarray(a, FP32), dtype=ml_dtypes.bfloat16)

    eye = np.eye(128, dtype=FP32)

    # encoder attention: only wx matters (softmax shift-invariance)
    wx = np.asarray(inp["enc_attn_w"], FP32)[0, 2 * H:]            # [10]

    # gate order permutation: torch (i,f,g,o) -> (f,i,o,g)
    perm = np.r_[64:128, 0:64, 192:256, 128:192]
    gsc = np.ones((256,), dtype=FP32); gsc[192:256] = 2.0

    eW, eU = np.asarray(inp["enc_Wih"], FP32), np.asarray(inp["enc_Whh"], FP32)
    eb = (np.asarray(inp["enc_bih"], FP32) + np.asarray(inp["enc_bhh"], FP32))[perm]
    encWihT = eW[perm].T * gsc[None, :]                            # [128,256]
    encWhhTb = np.vstack([eU[perm].T, eb[None]]) * gsc[None, :]    # [65,256]
    # per-partition ACT bias for the gate tanh: tanh(0.5*psum + bias)
    # m=0 psum is (f|i): bias=0.5*b ; m=1 is (o|g): [0.5*b_o ; b_g]
    biasE = np.zeros((128, 2), dtype=FP32)
    biasE[:, 0] = 0.5 * eb[0:128]
    biasE[0:64, 1] = 0.5 * eb[128:192]
    biasE[64:128, 1] = eb[192:256]

    # decoder attention MLP
    W1 = np.asarray(inp["dec_attn_w1"], FP32)                      # [64,192]
    W1h, W1c, W1x = W1[:, :64], W1[:, 64:128], W1[:, 128:]
    decW1h = np.hstack([W1h.T, W1h.T])                             # [64,128]
    decW1c = np.hstack([W1c.T, W1c.T])                             # [64,128]
    b1 = np.asarray(inp["dec_attn_b1"], FP32)
    decb1 = np.concatenate([b1, b1])[:, None]                      # [128,1] f32
    w2 = np.asarray(inp["dec_attn_w2"], FP32)[0]                   # [64]
    w2d = np.zeros((128, 2), dtype=FP32)
    w2d[:64, 0] = w2
    w2d[64:, 1] = w2
    wxI = np.hstack([np.eye(128, dtype=FP32) * wx[l] for l in range(L)])
    W1xE = np.zeros((64, 128), dtype=FP32); W1xE[:, 0:64] = W1x.T
    W1xO = np.zeros((64, 128), dtype=FP32); W1xO[:, 64:128] = W1x.T

    # decoder LSTM with folded fc layer
    dW, dU = np.asarray(inp["dec_Wih"], FP32), np.asarray(inp["dec_Whh"], FP32)
    fcw, fcb = np.asarray(inp["fc_w"], FP32), np.asarray(inp["fc_b"], FP32)
    W2c = (dW @ fcw[:, :64])[perm]                                 # [256,64]
    W2y = (dW @ fcw[:, 64:])[perm]                                 # [256,5]
    bp = (dW @ fcb + np.asarray(inp["dec_bih"], FP32)
          + np.asarray(inp["dec_bhh"], FP32))[perm]
    decWg1c = W2c.T * gsc[None, :]                                 # [64,256]
    decWg1h = dU[perm].T * gsc[None, :]                            # [64,256]
    decWg2yb = np.vstack([W2y.T, bp[None]]) * gsc[None, :]         # [6,256]
    biasD = np.zeros((128, 2), dtype=FP32)
    biasD[:, 0] = 0.5 * bp[0:128]
    biasD[0:64, 1] = 0.5 * bp[128:192]
    biasD[64:128, 1] = bp[192:256]

    fow, fob = np.asarray(inp["fcout_w"], FP32), np.asarray(inp["fcout_b"], FP32)

    # ---- pack weights into 2 tensors (one DMA each) ----
    def pack(arrs, dtype):
        cols = sum(a.shape[1] for a in arrs)
        buf = np.zeros((128, cols), dtype)
        o = 0
        offs = []
        for a in arrs:
            buf[: a.shape[0], o : o + a.shape[1]] = a
            offs.append(o)
            o += a.shape[1]
        return buf, offs

    wF, _ = pack([eye, decb1, biasE, biasD], FP32)                 # [128,133]
    bf_arrs = [
        np.eye(128, dtype=FP32),                                   # eyebf   0
        encWihT,                                                   # 128
        encWhhTb,                                                  # 384
        decW1h,                                                    # 640
        decW1c,                                                    # 768
        W1xE,                                                      # 896
        W1xO,                                                      # 1024
        decWg1c,                                                   # 1152
        decWg1h,                                                   # 1408
        decWg2yb,                                                  # 1664
        np.vstack([np.eye(64, dtype=FP32)] * 2),                   # eye2bf 1920
        w2d,                                                       # 1984
        wxI,                                                       # 1986
        fow[:, 64:].T,                                             # fcoutTc 3266
        fow[:, :64].T,                                             # fcoutTh 3271
        fob[None, :],                                              # fcoutb  3276
        np.ones((1, 1024), dtype=FP32),                            # ones    3281
        np.vstack([decW1c, decW1c]),                               # decW1cd 4305
    ]
    wB, offB = pack(bf_arrs, FP32)
    return {
        "wpackF": np.ascontiguousarray(wF),
        "wpackB": bf(wB),
    }


WB_COLS = 4433  # total bf16 pack columns (see _prep_weights order)


def build_module(BC):
    """Build the bass module for per-core batch BC (multiple of 256)."""
    nc = bacc.Bacc("TRN2", target_bir_lowering=False, debug=False)

    dt_in = {}

    def din(name, shape, dt=F32):
        dt_in[name] = nc.dram_tensor(name, list(shape), dt, kind="ExternalInput")
        return dt_in[name]

    din("x", (BC, L, D))
    din("y_hist", (BC, L, OUT))
    din("h0_enc", (BC, H))
    din("c0_enc", (BC, H))
    din("h0_dec", (BC, H))
    din("c0_dec", (BC, H))
    din("wpackF", (128, 133), F32)
    din("wpackB", (128, WB_COLS), BF16)

    out_d = nc.dram_tensor("out", [BC, OUT], F32, kind="ExternalOutput")

    with tile.TileContext(nc) as tc:
        _emit(nc, tc, dt_in, out_d, BC)
    nc.compile()
    return nc


def _emit(nc, tc, dd, out_d, BC):
    from contextlib import ExitStack

    CH = BC // 128                                  # 128-row batch chunks (8)
    NSPL = int(os.environ.get("NSPL", "4"))         # matmul-stage batch splits
    BW = BC // NSPL                                 # split width
    CHN = CH // NSPL                                # chunks per split
    NPR = NSPL // 2                                 # elementwise pair count
    PW = BC // NPR                                  # pair width

    ctx = ExitStack()
    with ctx:
        # ---------- persistent pools ----------
        wpool = ctx.enter_context(tc.tile_pool(name="weights", bufs=1))
        state = ctx.enter_context(tc.tile_pool(name="state", bufs=1))

        WF = wpool.tile([128, 133], F32, tag="wF")
        nc.sync.dma_start(WF[:], dd["wpackF"].ap())
        WB = wpool.tile([128, WB_COLS], BF16, tag="wB")
        nc.sync.dma_start(WB[:], dd["wpackB"].ap())

        eye = WF[:, 0:128]                 # f32 (setup transposes)
        decb1 = WF[:, 128:129]
        eyebf = WB[:, 0:128]
        encWihT = WB[:, 128:384]
        encWhhTb = WB[0:65, 384:640]
        decW1h = WB[0:64, 640:768]
        decW1c = WB[0:64, 768:896]
        decW1xE = WB[0:64, 896:1024]
        decW1xO = WB[0:64, 1024:1152]
        decWg1c = WB[0:64, 1152:1408]
        decWg1h = WB[0:64, 1408:1664]
        decWg2yb = WB[0:6, 1664:1920]
        eye2bf = WB[:, 1920:1984]
        w2dup = WB[:, 1984:1986]
        wxI = WB[:, 1986:3266]
        fcoutTc = WB[0:64, 3266:3271]
        fcoutTh = WB[0:64, 3271:3276]
        fcoutb = WB[0:1, 3276:3281]
        ones_bf = WB[0:1, 3281:3281 + BC]
        decW1cd = WB[:, 4305:4433]

        # persistent state tensors (all bf16); x_encT row 64 = ones (bias row)
        x_encT = state.tile([65, L + 1, BC], BF16, tag="x_encT")
        uT = state.tile([128, L, BC], BF16, tag="uT")
        xe_bm = state.tile([128, CH, H, L], BF16, tag="xe_bm")   # (c,h,l) l-inner
        pre_bf = state.tile([128, L // 2, BC], BF16, tag="pre_bf")
        zin_bf = state.tile([128, L // 2, BC], BF16, tag="zin_bf")
        z_bf = state.tile([128, L // 2, BC], BF16, tag="z_bf")
        yT6 = state.tile([6, L * BC], BF16, tag="yT6")           # row 5 = ones
        h_dec = state.tile([64, BC], BF16, tag="h_dec")
        c_dec = state.tile([64, BC], BF16, tag="c_dec")
        c_enc = state.tile([64, BC], BF16, tag="c_enc")
        ctxT = state.tile([64, BC], BF16, tag="ctxT")
        TG = state.tile([128, 2, BC], BF16, tag="TG")
        S1g = state.tile([128, BC], BF16, tag="S1g")
        Msb = state.tile([128, BC], BF16, tag="Msb")
        S2a = state.tile([64, BC], BF16, tag="S2a")
        thc = state.tile([64, BC], BF16, tag="thc")
        q2_bf = state.tile([128, BC], BF16, tag="q2_bf")

        SL = [slice(s * BW, (s + 1) * BW) for s in range(NSPL)]
        CS = [slice(s * CHN, (s + 1) * CHN) for s in range(NSPL)]
        PL = [slice(p * PW, (p + 1) * PW) for p in range(NPR)]
        PC = [slice(p * CHN * 2, (p + 1) * CHN * 2) for p in range(NPR)]

        # ---------- wavefront scheduler ----------
        def wavefront(stages):
            """stages: list over global stage k of [callable-or-None per
            split s]. Emit in order of (k+s), later stages first on ties,
            so each engine's in-order queue matches data-readiness order
            across splits and steps (software-pipelined recurrence)."""
            items = []
            for k, row in enumerate(stages):
                for s, fn in enumerate(row):
                    if fn is not None:
                        items.append((k + s, -k, len(items), fn))
            for _, _, _, fn in sorted(items, key=lambda x: (x[0], x[1], x[2])):
                fn()

        def per_split(f, t):
            return [(lambda t=t, s=s, f=f: f(t, s)) for s in range(NSPL)]

        def per_pair(f, t):
            # pair p covers splits 2p, 2p+1; key it at its later split
            row = [None] * NSPL
            for p in range(NPR):
                row[2 * p + 1] = (lambda t=t, p=p, f=f: f(t, p))
            return row

        # ---------- setup ----------
        with tc.tile_pool(name="setup_big", bufs=1) as sbp, \
             tc.tile_pool(name="setup_ps", bufs=2, space="PSUM") as sps, \
             tc.tile_pool(name="setup_ps2", bufs=1, space="PSUM") as sps2:

            # start every HBM load up front; transfers overlap on the DMA
            # engines while we compute on whatever has arrived
            x_sb = {}
            xr = dd["x"].ap().rearrange("(c p) l d -> p c l d", p=128)

            def x_dma(xq):
                cs = slice(xq * CH // 4, (xq + 1) * CH // 4)
                x_sb[xq] = sbp.tile([128, CH // 4, L, D], F32,
                                    tag="x_sb", name="x_sb", bufs=2)
                nc.sync.dma_start(x_sb[xq][:], xr[:, cs, :, :])

            for xq in range(2):
                x_dma(xq)
            y_bm = sbp.tile([128, CH, L * OUT], F32, tag="y_bm")
            nc.sync.dma_start(
                y_bm[:], dd["y_hist"].ap().rearrange("(c p) l o -> p c (l o)", p=128)
            )
            init_bm = {}
            for nm in ("h0_enc", "c0_enc", "h0_dec", "c0_dec"):
                tl = sbp.tile([128, CH, H], F32, tag=nm)
                nc.sync.dma_start(tl[:], dd[nm].ap().rearrange("(c p) h -> p c h", p=128))
                init_bm[nm] = tl

            # ones rows: x_encT bias row + yT6 row 5
            nc.sync.dma_start(
                x_encT[64:65, :, :],
                ones_bf.unsqueeze(1).broadcast_to([1, L + 1, BC]),
            )
            nc.sync.dma_start(
                yT6[5:6, :], ones_bf.unsqueeze(1).broadcast_to([1, L, BC]),
            )

            # y: transpose to [50, B] bf16, regroup to [6, L*B]
            psY = sps2.tile([50, CH, 128], F32, tag="psH")
            for c in range(CH):
                nc.tensor.transpose(psY[:, c, :], y_bm[:, c, :], eye)
            yT_all = sbp.tile([50, BC], BF16, tag="yT_all")
            nc.vector.tensor_copy(yT_all[:], psY[:].rearrange("p c b -> p (c b)"))
            for t in range(L):
                nc.sync.dma_start(
                    yT6[0:5, t * BC:(t + 1) * BC], yT_all[5 * t: 5 * t + 5, :]
                )

            # h0/c0: transpose, write bf16 states
            dsts = {
                "h0_enc": x_encT[0:64, 0, :], "c0_enc": c_enc[:, :],
                "h0_dec": h_dec[:, :], "c0_dec": c_dec[:, :],
            }
            engs = {"h0_enc": nc.vector, "c0_enc": nc.scalar,
                    "h0_dec": nc.vector, "c0_dec": nc.scalar}
            for nm in ("h0_enc", "c0_enc", "h0_dec", "c0_dec"):
                ps = sps2.tile([64, CH, 128], F32, tag="psH")
                for c in range(CH):
                    nc.tensor.transpose(ps[:, c, :], init_bm[nm][:, c, :], eye)
                e = engs[nm]
                if e is nc.scalar:
                    e.copy(dsts[nm], ps[:].rearrange("p c b -> p (c b)"))
                else:
                    e.tensor_copy(dsts[nm], ps[:].rearrange("p c b -> p (c b)"))


            # x -> bf16 (split across engines), score_x softmax, u, uT
            x_bf = sbp.tile([128, CH, L, D], BF16, tag="x_bf")
            for xq in range(4):
                for ci in range(CH // 4):
                    c = xq * CH // 4 + ci
                    eng = (nc.vector, nc.scalar)[ci]
                    if eng is nc.scalar:
                        nc.scalar.copy(x_bf[:, c, :, :], x_sb[xq][:, ci, :, :])
                    else:
                        eng.tensor_copy(x_bf[:, c, :, :], x_sb[xq][:, ci, :, :])
                if xq + 2 < 4:
                    x_dma(xq + 2)
            e_at = sbp.tile([128, CH, D], BF16, tag="e_at")
            Ssum = sbp.tile([128, CH], F32, tag="Ssum")
            psSX = {}
            for c in range(CH):
                t_sx = sps.tile([128, D], F32, tag="psSX")
                psSX[c] = t_sx
                for l in range(L):
                    nc.tensor.matmul(
                        t_sx[:], wxI[:, l * 128:(l + 1) * 128],
                        x_bf[:, c, l, :], start=(l == 0), stop=(l == L - 1),
                    )
            for c in range(CH):
                nc.scalar.activation(
                    e_at[:, c, :], psSX[c][:], AF.Exp, accum_out=Ssum[:, c:c + 1],
                )
            rS = sbp.tile([128, CH], F32, tag="rS")
            nc.vector.reciprocal(rS[:], Ssum[:])
            at0 = sbp.tile([128, CH, D], BF16, tag="at0")
            nc.vector.tensor_tensor(
                out=at0[:], in0=e_at[:],
                in1=rS[:].unsqueeze(2).broadcast_to([128, CH, D]), op=ALU.mult,
            )
            for c in range(CH):
                u_c = sbp.tile([128, L, D], BF16, tag="u_c")
                nc.vector.tensor_tensor(
                    out=u_c[:], in0=x_bf[:, c, :, :],
                    in1=at0[:, c, :].unsqueeze(1).broadcast_to([128, L, D]),
                    op=ALU.mult,
                )
                nc.sync.dma_start_transpose(
                    uT[:, :, c * 128:(c + 1) * 128], u_c[:].rearrange("p a b -> p (a b)"))

        # ---------- encoder loop + interleaved dec-pre (wavefront) ----------
        with tc.tile_pool(name="enc_g", bufs=4, space="PSUM") as pge, \
             tc.tile_pool(name="enc_c", bufs=2, space="PSUM") as pce, \
             tc.tile_pool(name="enc_xh", bufs=2) as pxh:

            psGs, psCs, psXEs, psPs = {}, {}, {}, {}

            def e_gates(t, s):
                sl = SL[s]
                g = pge.tile([128, 2, BW], F32, tag="g", name="ge")
                psGs[(t, s)] = g
                for m in (0, 1):
                    nc.tensor.matmul(
                        g[:, m, :], encWihT[:, m * 128:(m + 1) * 128],
                        uT[:, t, sl], start=True, stop=False)
                    nc.tensor.matmul(
                        g[:, m, :], encWhhTb[:, m * 128:(m + 1) * 128],
                        x_encT[:, t, sl], start=False, stop=True)

            def e_tg(t, s):
                nc.scalar.activation(
                    TG[:, :, SL[s]], psGs[(t, s)][:], AF.Tanh, scale=0.5)

            def e_s1g(t, s):
                sl = SL[s]
                nc.vector.tensor_scalar(
                    out=S1g[:, sl], in0=TG[:, 0, sl], scalar1=0.5, scalar2=0.5,
                    op0=ALU.mult, op1=ALU.add)

            def e_msb(t, s):
                sl = SL[s]
                nc.vector.tensor_tensor(
                    out=Msb[64:128, sl], in0=S1g[64:128, sl],
                    in1=TG[64:128, 1, sl], op=ALU.mult)
                nc.vector.tensor_tensor(
                    out=Msb[0:64, sl], in0=S1g[0:64, sl],
                    in1=c_enc[:, sl], op=ALU.mult)

            def e_psc(t, s):
                sl = SL[s]
                cps = pce.tile([64, BW], F32, tag="c", name="ce")
                psCs[s] = cps
                nc.tensor.matmul(cps[:], eye2bf, Msb[:, sl],
                                 start=True, stop=True)

            def e_tail(t, s):
                sl = SL[s]
                if s % 2 == 0:
                    nc.vector.tensor_copy(c_enc[:, sl], psCs[s][:])
                else:
                    nc.scalar.copy(c_enc[:, sl], psCs[s][:])
                nc.scalar.activation(thc[:, sl], psCs[s][:], AF.Tanh)
                nc.vector.tensor_scalar(
                    out=S2a[:, sl], in0=TG[0:64, 1, sl], scalar1=0.5,
                    scalar2=0.5, op0=ALU.mult, op1=ALU.add)

            def e_h(t, s):
                sl = SL[s]
                nc.vector.tensor_tensor(
                    out=x_encT[0:64, t + 1, sl], in0=S2a[:, sl],
                    in1=thc[:, sl], op=ALU.mult)

            def e_xec(t, s):
                # whole-width DMA transpose to a contiguous staging tile,
                # then a Pool (sbuf->sbuf) copy into the l-inner layout
                if s != NSPL - 1:
                    return
                xeh = pxh.tile([128, CH, H], BF16, tag="xeh", name="xeh")
                psXEs[t] = xeh
                nc.sync.dma_start_transpose(
                    xeh[:], x_encT[0:64, t + 1, :])

            def e_xep(t, s):
                if s != NSPL - 1:
                    return
                nc.gpsimd.tensor_copy(xe_bm[:, :, :, t], psXEs[t][:])

            def e_pre(j, p):
                # pre[:, j] = W1xE @ x_enc[2j+1] + W1xO @ x_enc[2j+2]
                # (E fills partition rows 0:64, O rows 64:128)
                pl = PL[p]
                pre_ps = pce.tile([128, PW], F32, tag="c")
                psPs[(j, p)] = pre_ps
                nc.tensor.matmul(
                    pre_ps[:], decW1xE, x_encT[0:64, 1 + 2 * j, pl],
                    start=True, stop=False)
                nc.tensor.matmul(
                    pre_ps[:], decW1xO, x_encT[0:64, 2 + 2 * j, pl],
                    start=False, stop=True)

            def e_prec(j, p):
                eng = (nc.vector, nc.scalar)[(2 * j + p) % 2]
                src = psPs[(j, p)][:]
                if eng is nc.scalar:
                    nc.scalar.copy(pre_bf[:, j, PL[p]], src)
                else:
                    eng.tensor_copy(pre_bf[:, j, PL[p]], src)

            EFNS = [(e_gates, per_split),
                    (e_tg, per_split), (e_s1g, per_split),
                    (e_msb, per_split), (e_psc, per_split), (e_tail, per_split),
                    (e_h, per_split), (e_xec, per_split), (e_xep, per_split)]
            stages = []
            for t in range(L):
                for f, kind in EFNS:
                    stages.append(kind(f, t))
                if t >= 2 and t % 2 == 0:
                    j = t // 2 - 1
                    stages.append(per_pair(
                        lambda tt, p, j=j: e_pre(j, p), t))
                    stages.append(per_pair(
                        lambda tt, p, j=j: e_prec(j, p), t))
            wavefront(stages)

            # last pre pair (j=4) after final encoder step
            for p in range(NPR):
                e_pre(4, p)
            for p in range(NPR):
                e_prec(4, p)

        # ---------- decoder loop (wavefront across splits+steps) ----------
        dwork = ctx.enter_context(tc.tile_pool(name="dwork", bufs=1))
        e_bf = dwork.tile([128, CH, L], BF16, tag="e_bf")
        at_bf = dwork.tile([128, CH, L], BF16, tag="at_bf")
        Ssm = dwork.tile([128, CH], F32, tag="Ssm")
        rSd = dwork.tile([128, CH], F32, tag="rSd")
        cm_bf = dwork.tile([128, CH, H, L], BF16, tag="cm_bf")
        s5_bf = dwork.tile([128, CH, H, L // 2], BF16, tag="s5_bf")
        t2_bf = dwork.tile([128, CH, H, 2], BF16, tag="t2_bf")
        ctx_bm = dwork.tile([128, CH, H], BF16, tag="ctx_bm")
        out_sb = dwork.tile([5, BC], F32, tag="out_sb")

        with tc.tile_pool(name="dec_q", bufs=2, space="PSUM") as pq, \
             tc.tile_pool(name="dec_g", bufs=4, space="PSUM") as pgd, \
             tc.tile_pool(name="dec_c", bufs=2, space="PSUM") as pcd:

            psQs, psGs, psCTs, psCs = {}, {}, {}, {}

            def d_q(t, s):
                # One [128,3,BW] psum tile per split/step: row 0 = q (later
                # reused for the z-score psums), rows 1:3 = the LSTM gates.
                # W1c@c via Msb ([W1c;W1c]@Msb == W1c@(eye2@Msb)) so q need
                # not wait for the c psum->sbuf copy of the previous step.
                sl = SL[s]
                qp = pq.tile([128, BW], F32, tag="q", name="qd")
                psQs[s] = qp
                qp = qp[:]
                if t == 0:
                    nc.tensor.matmul(qp, decW1c, c_dec[:, sl],
                                     start=True, stop=False)
                else:
                    nc.tensor.matmul(qp, decW1cd, Msb[:, sl],
                                     start=True, stop=False)
                nc.tensor.matmul(qp, decW1h, h_dec[:, sl],
                                 start=False, stop=True)

            def d_q2(t, s):
                nc.vector.tensor_copy(q2_bf[:, SL[s]], psQs[s][:])

            def d_zin(t, s):
                sl = SL[s]
                nc.vector.tensor_tensor(
                    out=zin_bf[:, :, sl], in0=pre_bf[:, :, sl],
                    in1=q2_bf[:, sl].unsqueeze(1)
                    .broadcast_to([128, L // 2, BW]), op=ALU.add)

            def d_z(t, s):
                sl = SL[s]
                nc.scalar.activation(
                    z_bf[:, :, sl], zin_bf[:, :, sl], AF.Tanh, bias=decb1)

            def d_sc(t, s):
                tp = psQs[s][:, 0:CHN * L].rearrange(
                    "p (c l) -> p c l", c=CHN)
                for ci in range(CHN):
                    c = s * CHN + ci
                    for j in range(L // 2):
                        nc.tensor.matmul(
                            tp[:, ci, 2 * j:2 * j + 2],
                            z_bf[:, j, c * 128:(c + 1) * 128], w2dup,
                            start=True, stop=True)

            def d_e(t, s):
                tp = psQs[s][:, 0:CHN * L].rearrange(
                    "p (c l) -> p c l", c=CHN)
                nc.scalar.activation(e_bf[:, CS[s], :], tp[:], AF.Exp)

            def d_sm(t, s):
                chs = CS[s]
                nc.vector.tensor_reduce(
                    out=Ssm[:, chs], in_=e_bf[:, chs, :], axis=AX.X,
                    op=ALU.add)
                nc.vector.reciprocal(rSd[:, chs], Ssm[:, chs])
                nc.vector.tensor_tensor(
                    out=at_bf[:, chs, :], in0=e_bf[:, chs, :],
                    in1=rSd[:, chs].unsqueeze(2)
                    .broadcast_to([128, CHN, L]), op=ALU.mult)

            def d_cm(t, s):
                chs = CS[s]
                nc.vector.tensor_tensor(
                    out=cm_bf[:, chs, :, :], in0=xe_bm[:, chs, :, :],
                    in1=at_bf[:, chs, :].unsqueeze(2)
                    .broadcast_to([128, CHN, H, L]), op=ALU.mult)

            def d_s5(t, s):
                chs = CS[s]
                nc.vector.tensor_tensor(
                    out=s5_bf[:, chs], in0=cm_bf[:, chs, :, 0:5],
                    in1=cm_bf[:, chs, :, 5:10], op=ALU.add)

            def d_ctt(t, s):
                # ctxT^T chunks: accumulate the 5 lag-pair transposes in PSUM
                cps = pcd.tile([128, CHN, 128], F32, tag="ct")
                psCTs[s] = cps
                for ci in range(CHN):
                    c = s * CHN + ci
                    for j in range(L // 2):
                        nc.tensor.matmul(
                            cps[64:128, ci, :], s5_bf[:, c, :, j], eyebf,
                            start=(j == 0), stop=(j == L // 2 - 1))

            def d_ctc(t, s):
                sl = SL[s]
                nc.vector.tensor_copy(
                    ctxT[:, sl],
                    psCTs[s][64:128, :, :].rearrange("p c b -> p (c b)"))

            def d_gates(t, s):
                sl = SL[s]
                g = pgd.tile([128, 2, BW], F32, tag="g", name="gd")
                psGs[s] = g
                for m in (0, 1):
                    nc.tensor.matmul(
                        g[:, m, :], decWg1c[:, m * 128:(m + 1) * 128],
                        ctxT[:, sl], start=True, stop=False)
                    nc.tensor.matmul(
                        g[:, m, :], decWg1h[:, m * 128:(m + 1) * 128],
                        h_dec[:, sl], start=False, stop=False)
                    nc.tensor.matmul(
                        g[:, m, :], decWg2yb[:, m * 128:(m + 1) * 128],
                        yT6[0:6, t * BC + sl.start: t * BC + sl.stop],
                        start=False, stop=True)

            def d_tg(t, s):
                nc.scalar.activation(
                    TG[:, :, SL[s]], psGs[s][:], AF.Tanh, scale=0.5)

            def d_s1g(t, s):
                sl = SL[s]
                nc.vector.tensor_scalar(
                    out=S1g[:, sl], in0=TG[:, 0, sl], scalar1=0.5,
                    scalar2=0.5, op0=ALU.mult, op1=ALU.add)

            def d_msb(t, s):
                sl = SL[s]
                nc.vector.tensor_tensor(
                    out=Msb[64:128, sl], in0=S1g[64:128, sl],
                    in1=TG[64:128, 1, sl], op=ALU.mult)
                nc.vector.tensor_tensor(
                    out=Msb[0:64, sl], in0=S1g[0:64, sl],
                    in1=c_dec[:, sl], op=ALU.mult)

            def d_psc(t, s):
                sl = SL[s]
                cps = pcd.tile([128, CHN, 128], F32, tag="ct")
                psCs[s] = cps[0:64, :, :].rearrange("p c b -> p (c b)")
                nc.tensor.matmul(psCs[s], eye2bf, Msb[:, sl],
                                 start=True, stop=True)

            def d_tail(t, s):
                sl = SL[s]
                nc.scalar.copy(c_dec[:, sl], psCs[s])
                nc.scalar.activation(thc[:, sl], psCs[s], AF.Tanh)
                nc.gpsimd.tensor_scalar(
                    out=S2a[:, sl], in0=TG[0:64, 1, sl], scalar1=0.5,
                    scalar2=0.5, op0=ALU.mult, op1=ALU.add)

            def d_h(t, s):
                sl = SL[s]
                nc.vector.tensor_tensor(
                    out=h_dec[:, sl], in0=S2a[:, sl], in1=thc[:, sl],
                    op=ALU.mult)

            DFNS = [(d_q, per_split), (d_q2, per_split),
                    (d_zin, per_split), (d_z, per_split),
                    (d_sc, per_split), (d_e, per_split),
                    (d_sm, per_split), (d_cm, per_split), (d_s5, per_split),
                    (d_ctt, per_split), (d_ctc, per_split),
                    (d_gates, per_split), (d_tg, per_split), (d_s1g, per_split),
                    (d_msb, per_split), (d_psc, per_split), (d_tail, per_split),
                    (d_h, per_split)]
            stages = []
            for t in range(L):
                for f, kind in DFNS:
                    stages.append(kind(f, t))
            wavefront(stages)

        # out = [ctx, h] @ fcout_w.T + fcout_b   (moving dim max 512)
        with tc.tile_pool(name="fin", bufs=1, space="PSUM") as pf:
            psO = pf.tile([5, BC], F32, tag="o")
            for hf in range(2):
                fsl = slice(hf * BC // 2, (hf + 1) * BC // 2)
                nc.tensor.matmul(psO[:, fsl], fcoutTc, ctxT[:, fsl],
                                 start=True, stop=False)
                nc.tensor.matmul(psO[:, fsl], fcoutTh, h_dec[:, fsl],
                                 start=False, stop=False)
                nc.tensor.matmul(psO[:, fsl], fcoutb, ones_bf[:, fsl],
                                 start=False, stop=True)
            nc.vector.tensor_copy(out_sb[:], psO[:])
            nc.sync.dma_start(out_d.ap().rearrange("b o -> o b"), out_sb[:])


_BUILD_CACHE = {}


def _get_module(BC):
    if BC not in _BUILD_CACHE:
        _BUILD_CACHE[BC] = build_module(BC)
    return _BUILD_CACHE[BC]


def kernel(**inputs):
    from concourse.bass_utils import run_bass_kernel_spmd

    B = inputs["x"].shape[0]
    BC = B // NCORES
    nc = _get_module(BC)
    prep = _prep_weights(inputs)

    data_keys = ["x", "y_hist", "h0_enc", "c0_enc", "h0_dec", "c0_dec"]
    in_maps = []
    for c in range(NCORES):
        sl = slice(c * BC, (c + 1) * BC)
        m = {k: np.ascontiguousarray(np.asarray(inputs[k], FP32)[sl]) for k in data_keys}
        m.update(prep)
        in_maps.append(m)

    res = run_bass_kernel_spmd(nc, in_maps, list(range(NCORES)))
    out = np.concatenate([r["out"] for r in res.results], axis=0)
    return np.ascontiguousarray(out, dtype=FP32)


if __name__ == "__main__":
    nc = build_module(1024)
    print("built OK")

